# revision 1
# baseline (speedup 1.0000x reference)
"""Trainium2 Bass kernel for nn_AttentionLayer (GAT-style masked attention).

Computes, for full inputs:
    h1 = emb_src @ W                      [8000, 128]
    g  = emb_dest @ (W @ W2)              [10000, 128]
    e  = g @ h1.T                         [10000, 8000]
    s  = lrelu(e, 0.2) * (1/sqrt(128))    masked to -inf where bias <= 0
    att = softmax(s, axis=1)
    out = att @ ft                        [10000, 1]   (ft = nan-cleaned feature_src)

Sharding: N_dest split across 8 NeuronCores (1250 rows each); emb_src /
feature_src / W / W2 replicated. No collectives. Softmax is computed
unnormalized (numer/denom) — no max-subtraction needed since |s| <= ~10.

Per-core device pipeline (per 128-row dest tile x 1000-col src slice):
    PE:     e_psum = gT.T @ h1T                       (bf16 x bf16 -> f32 PSUM)
    GPSIMD: amask  = (bias <= 0) * -1e30              (from streamed bias tile)
    DVE:    ms     = e_psum + amask
    ACT:    t      = Lrelu(SCALE * ms)  [alpha=0.2]
    ACT:    u      = Exp(t)             [accum_out -> denom partial]
    DVE:    ttr u * ft_bcast            [accum_out -> numer partial]
    out = numer / denom
"""
import os
import sys

sys.path.insert(0, "/opt/trn_rl_repo")

import numpy as np

_CACHE = {}

N_DEST, N_SRC, IN_DIM, HID = 10000, 8000, 256, 128
N_CORES = 8
ND = N_DEST // N_CORES            # 1250 dest rows per core
SCALE = float(1.0 / np.sqrt(np.float32(HID)))

# dest tiles per core: 9 x 128 + 98
DEST_TILES = [(i * 128, min(128, ND - i * 128)) for i in range((ND + 127) // 128)]
SRC_CHUNK = 500                   # matmul N (<= 512 = one PSUM bank of f32)
N_SRC_CHUNKS = N_SRC // SRC_CHUNK # 16
SS_W = 2 * SRC_CHUNK              # 1000-col superslice for ACT/DVE ops
N_SS = N_SRC // SS_W              # 8


def _build_nc():
    import concourse.bass as bass
    import concourse.tile as tile
    from concourse import bacc, mybir
    from concourse.masks import make_identity
    from contextlib import ExitStack

    F32 = mybir.dt.float32
    BF16 = mybir.dt.bfloat16
    FP16 = mybir.dt.float16
    AF = mybir.ActivationFunctionType
    OP = mybir.AluOpType

    nc = bacc.Bacc("TRN2", target_bir_lowering=False, debug=False,
                   num_devices=N_CORES)

    bias_t = nc.declare_dram_parameter("bias", [ND, N_SRC], F32, isOutput=False)
    dest_t = nc.declare_dram_parameter("emb_dest", [ND, IN_DIM], F32, isOutput=False)
    src_t = nc.declare_dram_parameter("emb_src", [N_SRC, IN_DIM], F32, isOutput=False)
    ft_t = nc.declare_dram_parameter("feature_src", [N_SRC, 1], F32, isOutput=False)
    w_t = nc.declare_dram_parameter("W", [IN_DIM, HID], F32, isOutput=False)
    w2_t = nc.declare_dram_parameter("W2", [HID, HID], F32, isOutput=False)
    out_t = nc.declare_dram_parameter("out", [ND, 1], F32, isOutput=True)

    with tile.TileContext(nc) as tc, ExitStack() as ctx:
        persist = ctx.enter_context(tc.tile_pool(name="persist", bufs=1))

        ident = persist.tile([128, 128], F32)
        make_identity(nc, ident)

        # ftbc: feature row broadcast across 128 partitions, then bf16
        ft_row = ft_t[:, :].rearrange("s one -> one s")   # [1, 8000] view
        ftbc = persist.tile([128, N_SRC], F32)
        nc.sync.dma_start(out=ftbc, in_=ft_row.to_broadcast([128, N_SRC]))
        ftbc_bf = persist.tile([128, N_SRC], BF16)
        nc.vector.tensor_copy(out=ftbc_bf, in_=ftbc)

        gt_tiles = []
        h1t_tiles = []

        with tc.tile_pool(name="pre_sb", bufs=3) as pre, \
             tc.tile_pool(name="pre_ps", bufs=2, space="PSUM") as pps:

            # ---- W chunks ([K=in_dim sub, M=hid]) natural + bf16
            w_sb = pre.tile([128, 2, HID], F32, tag="w_sb")
            for c in range(2):
                nc.sync.dma_start(out=w_sb[:, c, :], in_=w_t[128 * c:128 * (c + 1), :])
            w_bf = persist.tile([128, 2, HID], BF16)
            nc.vector.tensor_copy(out=w_bf, in_=w_sb)
            w2_sb = pre.tile([128, HID], F32, tag="w2_sb")
            nc.sync.dma_start(out=w2_sb, in_=w2_t[:, :])

            # ---- Wc = W @ W2, stored as lhsT chunks [K=in_dim sub, M=hid] bf16
            wc_bf = persist.tile([128, 2, HID], BF16)
            for c in range(2):
                ps_tr = pps.tile([128, 128], F32, tag="ps_a")
                nc.tensor.transpose(ps_tr, w_sb[:, c, :], ident)    # [hid, in_sub]
                wTc = pre.tile([128, 128], F32, tag="wTc")
                nc.scalar.copy(out=wTc, in_=ps_tr)
                ps_mm = pps.tile([128, HID], F32, tag="ps_b")
                nc.tensor.matmul(ps_mm, wTc, w2_sb, start=True, stop=True)
                nc.scalar.copy(out=wc_bf[:, c, :], in_=ps_mm)

            # ---- emb_dest -> destT (bf16, [in_sub, 2, dest]) -> gT tiles
            destT = pre.tile([128, 2, ND], BF16, tag="destT")
            for (r0, rn) in DEST_TILES:
                ed = pre.tile([128, IN_DIM], F32, tag="ed")
                nc.sync.dma_start(out=ed[:rn, :], in_=dest_t[r0:r0 + rn, :])
                for c in range(2):
                    ps_tr = pps.tile([128, 128], F32, tag="ps_a")
                    nc.tensor.transpose(ps_tr[:, :rn], ed[:rn, 128 * c:128 * (c + 1)],
                                        ident[:rn, :rn])
                    if c == 0:
                        nc.scalar.copy(out=destT[:, c, r0:r0 + rn], in_=ps_tr[:, :rn])
                    else:
                        nc.vector.tensor_copy(out=destT[:, c, r0:r0 + rn], in_=ps_tr[:, :rn])
            for ti, (r0, rn) in enumerate(DEST_TILES):
                ps_g = pps.tile([128, 128], F32, tag="ps_b")
                for c in range(2):
                    nc.tensor.matmul(ps_g[:, :rn], wc_bf[:, c, :],
                                     destT[:, c, r0:r0 + rn],
                                     start=(c == 0), stop=(c == 1))
                gt = persist.tile([128, 128], BF16, tag=f"gt{ti}")
                nc.scalar.copy(out=gt[:, :rn], in_=ps_g[:, :rn])
                gt_tiles.append(gt)

            # ---- emb_src -> srcT chunks -> h1T chunk tiles
            for j in range(N_SRC_CHUNKS):
                srcT = pre.tile([128, 2, SRC_CHUNK], BF16, tag="srcT")
                for k in range(4):                       # 4 x 125 src rows
                    s0 = j * SRC_CHUNK + k * 125
                    es = pre.tile([125, IN_DIM], F32, tag="es")
                    nc.sync.dma_start(out=es, in_=src_t[s0:s0 + 125, :])
                    for c in range(2):
                        ps_tr = pps.tile([128, 128], F32, tag="ps_a")
                        nc.tensor.transpose(ps_tr[:, :125],
                                            es[:, 128 * c:128 * (c + 1)],
                                            ident[:125, :125])
                        if (k + c) % 2 == 0:
                            nc.scalar.copy(out=srcT[:, c, 125 * k:125 * (k + 1)],
                                           in_=ps_tr[:, :125])
                        else:
                            nc.vector.tensor_copy(out=srcT[:, c, 125 * k:125 * (k + 1)],
                                                  in_=ps_tr[:, :125])
                ps_h = pps.tile([128, SRC_CHUNK], F32, tag="ps_b")
                for c in range(2):
                    nc.tensor.matmul(ps_h, w_bf[:, c, :], srcT[:, c, :],
                                     start=(c == 0), stop=(c == 1))
                h1t = persist.tile([128, SRC_CHUNK], BF16, tag=f"h1t{j}")
                nc.vector.tensor_copy(out=h1t, in_=ps_h)
                h1t_tiles.append(h1t)

        # ================= main loop =================
        with tc.tile_pool(name="mn_bias", bufs=4) as pbias, \
             tc.tile_pool(name="mn_mask", bufs=4) as pmask, \
             tc.tile_pool(name="mn_ms", bufs=3) as pms, \
             tc.tile_pool(name="mn_u", bufs=3) as pu, \
             tc.tile_pool(name="mn_small", bufs=2) as psm, \
             tc.tile_pool(name="mn_ps", bufs=3, space="PSUM") as mps:

            for ti, (r0, rn) in enumerate(DEST_TILES):
                gt = gt_tiles[ti]
                dpart = psm.tile([128, N_SS // 2], F32, tag="dpart")
                npart = psm.tile([128, N_SS // 2], F32, tag="npart")

                for p in range(N_SS // 2):
                    c0 = p * 2 * SS_W
                    btile = pbias.tile([128, 2 * SS_W], F32, tag="btile")
                    nc.sync.dma_start(out=btile[:rn, :],
                                      in_=bias_t[r0:r0 + rn, c0:c0 + 2 * SS_W])
                    amask = pmask.tile([128, 2 * SS_W], F32, tag="amask")
                    nc.vector.tensor_scalar(
                        out=amask[:rn, :], in0=btile[:rn, :],
                        scalar1=0.0, scalar2=-60000.0,
                        op0=OP.is_le, op1=OP.mult)

                    ms = pms.tile([128, 4, SRC_CHUNK], FP16, tag="ms")
                    for h in range(2):
                        ps_e = mps.tile([128, 2, 512], F32, tag="ps_e")
                        for q in range(2):
                            nc.tensor.matmul(
                                ps_e[:rn, q, 0:SRC_CHUNK],
                                gt[:, :rn], h1t_tiles[4 * p + 2 * h + q],
                                start=True, stop=True)
                        nc.vector.tensor_add(
                            ms[:rn, 2 * h:2 * h + 2, :],
                            ps_e[:rn, :, 0:SRC_CHUNK],
                            amask[:rn, h * SS_W:(h + 1) * SS_W].rearrange(
                                "p (b c) -> p b c", b=2))

                    msf = ms[:rn].rearrange("p b c -> p (b c)")
                    t0 = pms.tile([128, 2 * SS_W], FP16, tag="t0")
                    nc.vector.tensor_scalar_mul(t0[:rn, :], msf, 0.2)
                    t = pms.tile([128, 2 * SS_W], FP16, tag="t")
                    nc.vector.tensor_max(t[:rn, :], msf, t0[:rn, :])
                    u = pu.tile([128, 2 * SS_W], BF16, tag="u")
                    nc.scalar.activation(out=u[:rn, :], in_=t[:rn, :],
                                         func=AF.Exp, scale=SCALE,
                                         accum_out=dpart[:rn, p:p + 1])
                    prod = pu.tile([128, 2 * SS_W], BF16, tag="prod")
                    nc.vector.tensor_mul(prod[:rn, :], u[:rn, :],
                                         ftbc_bf[:rn, c0:c0 + 2 * SS_W])
                    scrap = pu.tile([128, 2 * SS_W], BF16, tag="scrap")
                    nc.scalar.activation(out=scrap[:rn, :], in_=prod[:rn, :],
                                         func=AF.Copy,
                                         accum_out=npart[:rn, p:p + 1])

                den = psm.tile([128, 1], F32, tag="den")
                nc.vector.tensor_reduce(den[:rn, :], dpart[:rn, :],
                                        axis=mybir.AxisListType.X, op=OP.add)
                num = psm.tile([128, 1], F32, tag="num")
                nc.vector.tensor_reduce(num[:rn, :], npart[:rn, :],
                                        axis=mybir.AxisListType.X, op=OP.add)
                rden = psm.tile([128, 1], F32, tag="rden")
                nc.vector.reciprocal(out=rden[:rn, :], in_=den[:rn, :])
                o = psm.tile([128, 1], F32, tag="o")
                nc.vector.tensor_mul(o[:rn, :], num[:rn, :], rden[:rn, :])
                nc.sync.dma_start(out=out_t[r0:r0 + rn, :], in_=o[:rn, :])

    nc.compile()
    return nc


def _get_nc():
    if "nc" not in _CACHE:
        _CACHE["nc"] = _build_nc()
    return _CACHE["nc"]


def kernel(bias, emb_dest, emb_src, feature_src, W, W2, _trace=False):
    from concourse.bass_utils import run_bass_kernel_spmd

    bias = np.ascontiguousarray(bias, dtype=np.float32)
    emb_dest = np.ascontiguousarray(emb_dest, dtype=np.float32)
    emb_src = np.ascontiguousarray(emb_src, dtype=np.float32)
    ft = np.ascontiguousarray(feature_src, dtype=np.float32)
    W = np.ascontiguousarray(W, dtype=np.float32)
    W2 = np.ascontiguousarray(W2, dtype=np.float32)

    nan_ind = np.isnan(ft.reshape(-1))
    if nan_ind.any():
        # NaN source features: zero the feature and mask out the column
        # (matches reference semantics). Never hit for randn inputs.
        ft = np.where(np.isnan(ft), 0.0, ft)
        bias = np.where(nan_ind.reshape(1, -1), -1.0, bias)

    nc = _get_nc()
    in_maps = []
    for i in range(N_CORES):
        r0 = i * ND
        in_maps.append({
            "bias": bias[r0:r0 + ND],
            "emb_dest": emb_dest[r0:r0 + ND],
            "emb_src": emb_src,
            "feature_src": ft,
            "W": W,
            "W2": W2,
        })
    res = run_bass_kernel_spmd(nc, in_maps, list(range(N_CORES)),
                               trace=_trace)
    out = np.concatenate([res.results[i]["out"] for i in range(N_CORES)], axis=0)
    if _trace:
        return out, res
    return out



# revision 14
# speedup vs baseline: 1.8913x; 1.8913x over previous
"""Trainium2 Bass kernel for nn_AttentionLayer (GAT-style masked attention).

Computes, for full inputs:
    h1 = emb_src @ W                      [8000, 128]
    g  = emb_dest @ (W @ W2)              [10000, 128]
    e  = g @ h1.T                         [10000, 8000]
    s  = lrelu(e, 0.2) * (1/sqrt(128))    masked to -inf where bias <= 0
    att = softmax(s, axis=1)
    out = att @ ft                        [10000, 1]   (ft = nan-cleaned feature_src)

Sharding: N_dest split across 8 NeuronCores (1250 rows each); emb_src /
feature_src / W / W2 replicated. No collectives. Softmax is unnormalized
(numer/denom) — no max subtraction needed since |scale*lrelu(e)| <= ~15.

Layout: TRANSPOSED on-device — scores are computed as e.T tiles
[src=partition, dest=free] so that BOTH softmax reductions (denominator
sum(u) and numerator sum(u*ft)) run on the Tensor engine as accumulating
matmuls with lhsT = [ones | ft_chunk], leaving DVE/ACT/Pool only the
mask + LeakyReLU + exp elementwise chain. Host-side staging is layout
only: bias is staged transposed+tile-contiguous [63,128,1280] (one
contiguous 640KB DMA per src tile), embeddings transposed, ft staged
partition-major [128,63].

Per-core device pipeline, per src tile s (63 tiles):
    DMA:   btT    = bias.T tile              [128,1280] f32
    POOL:  amask  = (btT <= 0) * 3e4         fp16
    PE:    psE    = SCALE * h1T_s.T @ gts    f32 PSUM (3 bank chunks)
    DVE:   s6m    = 0.6*psE - amask          (scalar_tensor_tensor, fp16)
    ACT/POOL: c   = (2/3)*|s6m|              (split by columns)
    DVE:   t      = s6m + c                  (2x; = lrelu(scale*e) - mask)
    ACT:   u      = Exp(t) -> bf16
    PE:    psR   += [ones | ft_s].T @ u      (accumulated over all 63 s)
Final: out = psR[num] / psR[den] per dest column, one 5KB DMA.

LeakyReLU identity: lrelu(x) = 0.6x + 0.4|x|; with m in {0, 3e4},
s6m = 0.6x - m  =>  s6m + (2/3)|s6m| = lrelu(x) exactly when m=0, and
<= -m/3 + 0.2|x| (hugely negative => exp==0) when masked.
"""
import os
import sys

sys.path.insert(0, "/opt/trn_rl_repo")

import numpy as np

_CACHE = {}

N_DEST, N_SRC, IN_DIM, HID = 10000, 8000, 256, 128
N_CORES = 8
ND = N_DEST // N_CORES            # 1250 dest rows per core
NDP = 1280                        # dest padded (free axis of transposed tiles)
NSP = 8064                        # src padded to 63 full 128-row tiles
NST = NSP // 128                  # 63 src tiles
SCALE = float(1.0 / np.sqrt(np.float32(HID)))

HC = 1000                         # h1T build chunk width
N_HC = N_SRC // HC                # 8

CHK = [(0, 512), (512, 512), (1024, 256)]   # dest chunks (PSUM banks)
ABS_ACT = 1120                    # |s6m| split: ACT [0:1120], POOL [1120:1280]
MBIG = 30000.0                    # mask magnitude (fp16-safe)


def _build_nc():
    import concourse.bass as bass
    import concourse.tile as tile
    from concourse import bacc, mybir
    from concourse.masks import make_identity
    from contextlib import ExitStack

    F32 = mybir.dt.float32
    BF16 = mybir.dt.bfloat16
    FP16 = mybir.dt.float16
    AF = mybir.ActivationFunctionType
    OP = mybir.AluOpType

    nc = bacc.Bacc("TRN2", target_bir_lowering=False, debug=False,
                   num_devices=N_CORES)

    bias_t = nc.declare_dram_parameter("biasT", [NST, 128, NDP], F32,
                                       isOutput=False)
    destT_t = nc.declare_dram_parameter("emb_destT", [IN_DIM, ND], F32,
                                        isOutput=False)
    srcT_t = nc.declare_dram_parameter("emb_srcT", [IN_DIM, N_SRC], F32,
                                       isOutput=False)
    ftc_t = nc.declare_dram_parameter("ft_cols", [128, NST], F32,
                                      isOutput=False)
    w_t = nc.declare_dram_parameter("W", [IN_DIM, HID], F32, isOutput=False)
    w2_t = nc.declare_dram_parameter("W2", [HID, HID], F32, isOutput=False)
    out_t = nc.declare_dram_parameter("out", [1, ND], F32, isOutput=True)

    with tile.TileContext(nc) as tc, ExitStack() as ctx:
        persist = ctx.enter_context(tc.tile_pool(name="persist", bufs=1))

        # persistent tiles
        gts = persist.tile([128, NDP], BF16)      # SCALE * g.T  [hid, dest]
        h1t = persist.tile([128, NSP], BF16)      # h1.T         [hid, src]
        ftw = persist.tile([128, 2 * NST], BF16)  # per-src-tile [ones | ft]

        with tc.tile_pool(name="pre_sb", bufs=2) as pre, \
             tc.tile_pool(name="pre_big", bufs=2) as preb, \
             tc.tile_pool(name="pre_ps", bufs=2, space="PSUM") as pps:

            ident = pre.tile([128, 128], F32, tag="ident")
            make_identity(nc, ident)

            # ---- W chunks ([K=in_sub, M=hid]) + bf16
            w_sb = pre.tile([128, 2, HID], F32, tag="w_sb")
            for c in range(2):
                nc.sync.dma_start(out=w_sb[:, c, :],
                                  in_=w_t[128 * c:128 * (c + 1), :])
            w_bf = pre.tile([128, 2, HID], BF16, tag="w_bf")
            nc.vector.tensor_copy(out=w_bf, in_=w_sb)
            w2_sb = pre.tile([128, HID], F32, tag="w2_sb")
            nc.sync.dma_start(out=w2_sb, in_=w2_t[:, :])

            # ---- Wc = W @ W2 as [K=in_sub, M=hid] chunks, bf16
            wc_bf = pre.tile([128, 2, HID], BF16, tag="wc_bf")
            for c in range(2):
                ps_tr = pps.tile([128, 128], F32, tag="ps_a")
                nc.tensor.transpose(ps_tr, w_sb[:, c, :], ident)  # [hid, in_sub]
                wTc = pre.tile([128, 128], F32, tag="wTc")
                nc.scalar.copy(out=wTc, in_=ps_tr)
                ps_mm = pps.tile([128, HID], F32, tag="ps_b")
                nc.tensor.matmul(ps_mm, wTc, w2_sb, start=True, stop=True)
                nc.scalar.copy(out=wc_bf[:, c, :], in_=ps_mm)

            # ---- emb_destT -> bf16 -> gts (= SCALE * Wc.T @ emb_dest.T)
            dsb = preb.tile([128, 2, ND], F32, tag="dsb")
            for c in range(2):
                nc.sync.dma_start(out=dsb[:, c, :],
                                  in_=destT_t[128 * c:128 * (c + 1), :])
            dbf = preb.tile([128, 2, NDP], BF16, tag="dbf")
            nc.gpsimd.memset(dbf[:, :, ND:NDP], 0.0)
            nc.vector.tensor_copy(out=dbf[:, 0, :ND], in_=dsb[:, 0, :])
            nc.scalar.copy(out=dbf[:, 1, :ND], in_=dsb[:, 1, :])
            for d0 in range(0, NDP, 512):
                dw = min(512, NDP - d0)
                ps_g = pps.tile([128, 512], F32, tag="ps_b")
                for c in range(2):
                    nc.tensor.matmul(ps_g[:, :dw], wc_bf[:, c, :],
                                     dbf[:, c, d0:d0 + dw],
                                     start=(c == 0), stop=(c == 1))
                nc.scalar.activation(out=gts[:, d0:d0 + dw], in_=ps_g[:, :dw],
                                     func=AF.Copy, scale=SCALE)

            # ---- emb_srcT -> bf16 -> h1T (= W.T @ emb_src.T), streamed
            nc.gpsimd.memset(h1t[:, N_SRC:NSP], 0.0)
            for j in range(N_HC):
                j0 = j * HC
                ssb = preb.tile([128, 2, HC], F32, tag="ssb")
                for c in range(2):
                    nc.sync.dma_start(
                        out=ssb[:, c, :],
                        in_=srcT_t[128 * c:128 * (c + 1), j0:j0 + HC])
                sbf = preb.tile([128, 2, HC], BF16, tag="sbf")
                nc.vector.tensor_copy(out=sbf[:, 0, :], in_=ssb[:, 0, :])
                nc.scalar.copy(out=sbf[:, 1, :], in_=ssb[:, 1, :])
                for half in range(2):
                    ps_h = pps.tile([128, 512], F32, tag="ps_b")
                    for c in range(2):
                        nc.tensor.matmul(
                            ps_h[:, :500], w_bf[:, c, :],
                            sbf[:, c, half * 500:half * 500 + 500],
                            start=(c == 0), stop=(c == 1))
                    if half == 0:
                        nc.scalar.copy(out=h1t[:, j0:j0 + 500],
                                       in_=ps_h[:, :500])
                    else:
                        nc.vector.tensor_copy(out=h1t[:, j0 + 500:j0 + HC],
                                              in_=ps_h[:, :500])

            # ---- ftw: per src tile s, columns [2s, 2s+1] = [ones, ft_s]
            ftc_sb = pre.tile([128, NST], F32, tag="ftc_sb")
            nc.sync.dma_start(out=ftc_sb, in_=ftc_t[:, :])
            ftw_v = ftw[:, :].rearrange("p (s two) -> p s two", two=2)
            nc.gpsimd.memset(ftw_v[:, :, 0], 1.0)
            nc.vector.tensor_copy(out=ftw_v[:, :, 1], in_=ftc_sb)

        # ================= main loop =================
        with tc.tile_pool(name="mn_bias", bufs=3) as pbias, \
             tc.tile_pool(name="mn_mask", bufs=3) as pmask, \
             tc.tile_pool(name="mn_t", bufs=3) as pt, \
             tc.tile_pool(name="mn_u", bufs=3) as pu, \
             tc.tile_pool(name="mn_small", bufs=1) as psm, \
             tc.tile_pool(name="mn_acc", bufs=1, space="PSUM") as pacc, \
             tc.tile_pool(name="mn_ps", bufs=2, space="PSUM") as mps:

            psR = pacc.tile([128, 512], F32)  # rows 32k: denom, 32k+1: numer

            for s in range(NST):
                btT = pbias.tile([128, NDP], F32, tag="btT")
                nc.sync.dma_start(out=btT, in_=bias_t[s])
                amask = pmask.tile([128, NDP], FP16, tag="amask")
                nc.scalar.activation(out=amask, in_=btT, func=AF.Relu,
                                     scale=-1e30)

                psE = mps.tile([128, 1536], F32, tag="psE")
                for (o, w) in CHK:
                    nc.tensor.matmul(psE[:, o:o + w],
                                     h1t[:, 128 * s:128 * (s + 1)],
                                     gts[:, o:o + w], start=True, stop=True)

                sm = pt.tile([128, NDP], FP16, tag="sm")
                nc.vector.scalar_tensor_tensor(
                    out=sm, in0=psE[:, :NDP], scalar=1.0, in1=amask,
                    op0=OP.mult, op1=OP.subtract)
                y = pt.tile([128, NDP], FP16, tag="y")
                nc.vector.tensor_scalar_mul(y, sm, 0.2)
                t = pt.tile([128, NDP], FP16, tag="t")
                nc.vector.tensor_max(t, sm, y)
                u = pu.tile([128, NDP], BF16, tag="u")
                nc.scalar.activation(out=u, in_=t, func=AF.Exp)

                for k, (o, w) in enumerate(CHK):
                    nc.tensor.matmul(psR[32 * k:32 * k + 2, :w],
                                     ftw[:, 2 * s:2 * s + 2], u[:, o:o + w],
                                     start=(s == 0), stop=(s == NST - 1))

            # ---- finals: out = numer / denom, one row DMA
            rsb = psm.tile([66, 512], F32, tag="rsb")
            nc.scalar.copy(out=rsb, in_=psR[:66, :])
            osb = psm.tile([1, NDP], F32, tag="osb")
            dbg = os.environ.get("KDBG", "")
            for k, (o, w) in enumerate(CHK):
                nrow = psm.tile([1, 512], F32, tag=f"nrow{k}")
                nc.sync.dma_start(out=nrow[:, :w],
                                  in_=rsb[32 * k + 1:32 * k + 2, :w])
                if dbg == "den":
                    nc.vector.tensor_copy(out=osb[:, o:o + w],
                                          in_=rsb[32 * k:32 * k + 1, :w])
                    continue
                if dbg == "num":
                    nc.vector.tensor_copy(out=osb[:, o:o + w],
                                          in_=nrow[:, :w])
                    continue
                rec = psm.tile([1, 512], F32, tag=f"rec{k}")
                nc.vector.reciprocal(out=rec[:, :w],
                                     in_=rsb[32 * k:32 * k + 1, :w])
                nc.vector.tensor_mul(osb[:, o:o + w], nrow[:, :w],
                                     rec[:, :w])
            nc.sync.dma_start(out=out_t[:, :], in_=osb[:, :ND])

    nc.compile()
    return nc


def _get_nc():
    if "nc" not in _CACHE:
        _CACHE["nc"] = _build_nc()
    return _CACHE["nc"]


def kernel(bias, emb_dest, emb_src, feature_src, W, W2, _trace=False):
    from concourse.bass_utils import run_bass_kernel_spmd

    bias = np.ascontiguousarray(bias, dtype=np.float32)
    emb_dest = np.ascontiguousarray(emb_dest, dtype=np.float32)
    emb_src = np.ascontiguousarray(emb_src, dtype=np.float32)
    ft = np.ascontiguousarray(feature_src, dtype=np.float32).reshape(-1)
    W = np.ascontiguousarray(W, dtype=np.float32)
    W2 = np.ascontiguousarray(W2, dtype=np.float32)

    nan_ind = np.isnan(ft)
    if nan_ind.any():
        # NaN source features: zero the feature and mask out the column
        # (matches reference semantics). Never hit for randn inputs.
        ft = np.where(nan_ind, 0.0, ft)
        bias = np.where(nan_ind.reshape(1, -1), -1.0, bias)

    srcT = np.ascontiguousarray(emb_src.T)          # [256, 8000]
    ftp = np.zeros(NSP, dtype=np.float32)
    ftp[:N_SRC] = ft
    ft_cols = np.ascontiguousarray(ftp.reshape(NST, 128).T)  # [128, 63]

    nc = _get_nc()
    in_maps = []
    for i in range(N_CORES):
        r0 = i * ND
        slabT = np.zeros((NSP, NDP), dtype=np.float32)
        slabT[:N_SRC, :ND] = bias[r0:r0 + ND].T
        in_maps.append({
            "biasT": slabT.reshape(NST, 128, NDP),
            "emb_destT": np.ascontiguousarray(emb_dest[r0:r0 + ND].T),
            "emb_srcT": srcT,
            "ft_cols": ft_cols,
            "W": W,
            "W2": W2,
        })
    res = run_bass_kernel_spmd(nc, in_maps, list(range(N_CORES)),
                               trace=_trace)
    out = np.concatenate(
        [res.results[i]["out"].reshape(ND, 1) for i in range(N_CORES)], axis=0)
    if _trace:
        return out, res
    return out
